# revision 1
# baseline (speedup 1.0000x reference)
"""BiLSTM-CRF loss kernel for Trainium2, 8-core data parallel.

Per-core (batch shard of 32, both LSTM directions as independent chains):
  P0: dma_gather embeddings (bf16, transposed layout: E on partitions)
  P1: input projections x @ W_ih.T + b -> zin (bf16, DRAM scratch)
  P2: 128 LSTM steps; fwd and bwd emitted per step as separate instruction
      chains so the engines pipeline across directions; h transposed per step
      via PE into hT buffers (feature-major) feeding the next step's matmul
      lhsT and the emission matmuls
  P3: emission matmuls + gold-path dot (tensor_tensor_reduce) + exp(em)
  P4: CRF forward pass in scaled linear space with an absorbing 77th tag for
      variable lengths; final log + reductions -> per-core partial sums
Host combines the 8 partial sums into the scalar loss.
"""

import numpy as np
import ml_dtypes

import concourse.bass as bass
import concourse.mybir as mybir
from concourse.tile import TileContext
from concourse import library_config
from concourse.vector_clock import ScopedClock

N_CORES = 8
B, S, E, HD, T, V = 256, 128, 512, 256, 76, 30000
BC = B // N_CORES          # 32 batch per core
G4 = 4 * HD                # 1024 gates
TA = T + 1                 # 77 tags with absorber
NTOK = S * BC              # 4096 tokens per direction per core

dt = mybir.dt
F32, BF16, I16 = dt.float32, dt.bfloat16, dt.int16
AF = mybir.ActivationFunctionType
ALU = mybir.AluOpType

# ---------------------------------------------------------------- tile patch
# This walrus build rejects >1 sem wait on CTRL-class (Drain/NoOp)
# instructions; split the Tile tail-drain waits across preceding NOPs.
_MAX_WAITS = 1


_WAIT_LIMITS = {}


def _split_excess_waits(nc):
    """Non-DMA instructions accept only one sem wait on this walrus build;
    move excess waits onto NOPs spliced in front (same engine, same order)."""
    for f in nc.m.functions:
        stack = list(f.blocks)
        while stack:
            bb = stack.pop()
            for sub in getattr(bb, "blocks", []) or []:
                stack.append(sub)
            insts = getattr(bb, "instructions", None)
            if not insts:
                continue
            newlist = []
            changed = False
            for inst in insts:
                si = inst.sync_info
                lim = _WAIT_LIMITS.get(type(inst).__name__, 1)
                if si is not None and si.on_wait and len(si.on_wait) > lim:
                    waits = list(si.on_wait)
                    si.on_wait = waits[-lim:]
                    for w in waits[:-lim]:
                        nop = mybir.InstNoOp(
                            name=f"I-wsplit{nc.next_id()}", ins=[], outs=[],
                            engine=inst.engine,
                            sync_info=mybir.SyncInfo(on_wait=[w], on_update=[]),
                        )
                        newlist.append(nop)
                    changed = True
                newlist.append(inst)
            if changed:
                insts[:] = newlist


def _patched_drain_and_barrier(self, tick_clock, wait_clock):
    nc = self.nc
    _split_excess_waits(nc)
    nops = [nc.sync.nop(nofuse=True, hint=f"waitsplit{i}") for i in range(16)]
    drain_inst = nc.sync.drain()
    wait_clock.add_sem_waits(
        drain_inst.ins, ScopedClock({None: tick_clock.global_clock})
    )
    si = drain_inst.ins.sync_info
    if si is not None and si.on_wait and len(si.on_wait) > _MAX_WAITS:
        waits = list(si.on_wait)
        chunks = [waits[i:i + _MAX_WAITS] for i in range(0, len(waits), _MAX_WAITS)]
        si.on_wait = chunks[-1]
        assert len(chunks) - 1 <= len(nops), "too many wait chunks"
        for i, ch in enumerate(chunks[:-1]):
            ni = nops[i].ins
            if ni.sync_info is None:
                ni.sync_info = mybir.SyncInfo(on_wait=ch, on_update=[])
            else:
                ni.sync_info.on_wait = list(ni.sync_info.on_wait) + ch
    nc.all_engine_barrier()
    assert self.sems is not None
    popped = nc._tile_sem_poison_stack.pop()
    assert popped is self._sem_poison
    allsems = list(self.sems.allocated().values())
    for i in range(0, len(allsems), 8):
        nc.clear_and_free_semaphores(allsems[i:i + 8])
    nc.all_engine_barrier()


def apply_tile_patch():
    TileContext._drain_and_barrier = _patched_drain_and_barrier


# ---------------------------------------------------------------- builder
def build_nc():
    apply_tile_patch()
    nc = bass.Bass("TRN2", target_bir_lowering=False, debug=False,
                   num_devices=N_CORES)

    xt_d = nc.dram_tensor("xt", [2, 128, 4, NTOK], BF16, kind="ExternalInput")
    wih = nc.dram_tensor("wih", [2, 128, 4, G4], BF16, kind="ExternalInput")
    whh = nc.dram_tensor("whh", [2, 128, 2, G4], BF16, kind="ExternalInput")
    wout = nc.dram_tensor("wout", [128, 4, T], BF16, kind="ExternalInput")
    # per-dir combined bias b_ih+b_hh (gate-reordered), replicated over 128 rows
    biasr = nc.dram_tensor("biasr", [2, 128, G4], BF16, kind="ExternalInput")
    h0t = nc.dram_tensor("h0t", [128, 2, 2 * BC], BF16, kind="ExternalInput")
    c0 = nc.dram_tensor("c0", [2 * BC, HD], F32, kind="ExternalInput")
    ident = nc.dram_tensor("ident", [128, 96], BF16, kind="ExternalInput")
    # tables: [trans(0:76) | start(76) | end(77) | bout(78) | negkappa(79)]
    tables = nc.dram_tensor("tables", [T, 80], F32, kind="ExternalInput")
    gcnt = nc.dram_tensor("gcnt", [T, 79], F32, kind="ExternalInput")
    ohm = nc.dram_tensor("ohm", [T, NTOK], BF16, kind="ExternalInput")
    vmask = nc.dram_tensor("vmask", [T, NTOK], BF16, kind="ExternalInput")
    padrow = nc.dram_tensor("padrow", [1, NTOK], F32, kind="ExternalInput")
    absrow = nc.dram_tensor("absrow", [1, 80], F32, kind="ExternalInput")
    out_d = nc.dram_tensor("out", [1, 2], F32, kind="ExternalOutput")
    zin_d = nc.dram_tensor("zin_scratch", [2, S // 4, 128, G4], BF16,
                           kind="Internal")

    with TileContext(nc) as tc:
        with (
            tc.tile_pool(name="const", bufs=1) as cpool,
            tc.tile_pool(name="hbuf", bufs=1) as hpool,
            tc.tile_pool(name="work", bufs=3) as wpool,
            tc.tile_pool(name="state", bufs=3) as spool,
            tc.tile_pool(name="mmps", bufs=2, space="PSUM") as mmps,
            tc.tile_pool(name="zups", bufs=1, space="PSUM") as zups,
            tc.tile_pool(name="smps", bufs=2, space="PSUM") as smps,
        ):
            # ---- constants / small inputs into SBUF
            wih_sb = cpool.tile([128, 2, 4, G4], BF16)
            nc.sync.dma_start(wih_sb[:], wih.ap().rearrange("d p c g -> p d c g"))
            whh_sb = cpool.tile([128, 2, 2, G4], BF16)
            nc.sync.dma_start(whh_sb[:], whh.ap().rearrange("d p c g -> p d c g"))
            wout_sb = cpool.tile([128, 4, T], BF16)
            nc.sync.dma_start(wout_sb[:], wout[:])
            bias_sb = cpool.tile([128, 2, G4], BF16)
            nc.sync.dma_start(bias_sb[:], biasr.ap().rearrange("d p g -> p d g"))
            h0t_sb = cpool.tile([128, 2, 2 * BC], BF16)
            nc.sync.dma_start(h0t_sb[:], h0t[:])
            ident_sb = cpool.tile([128, 96], BF16)
            nc.sync.dma_start(ident_sb[:], ident[:])
            tab_sb = cpool.tile([T, 80], F32)
            nc.sync.dma_start(tab_sb[:], tables[:])
            gcnt_sb = cpool.tile([T, 79], F32)
            nc.sync.dma_start(gcnt_sb[:], gcnt[:])
            # persistent big buffers
            hts = {0: hpool.tile([128, 2, NTOK], BF16, tag="hft", name="hft"),
                   1: hpool.tile([128, 2, NTOK], BF16, tag="hbt", name="hbt")}
            em_sb = hpool.tile([TA, NTOK], F32, tag="em")

            # ---- P0 + P1 in a released pool
            with tc.tile_pool(name="xg", bufs=1) as xpool:
                xg = {0: xpool.tile([128, 4, NTOK], BF16, tag="xg0", name="xg0"),
                      1: xpool.tile([128, 4, NTOK], BF16, tag="xg1", name="xg1")}
                for d in range(2):
                    nc.sync.dma_start(xg[d][:], xt_d.ap()[d])

                # token block of 128 = 4 steps; PSUM [128, 512] x2 slices
                for d in range(2):
                    for tb in range(NTOK // 128):     # 32 blocks
                        stg = wpool.tile([128, G4], BF16, tag="zstage")
                        for sl in range(2):
                            ps = mmps.tile([128, 512], F32, tag="mm")
                            for c in range(4):
                                nc.tensor.matmul(
                                    ps[:],
                                    xg[d][:, c, tb * 128:(tb + 1) * 128],
                                    wih_sb[:, d, c, sl * 512:(sl + 1) * 512],
                                    start=(c == 0), stop=(c == 3),
                                )
                            nc.vector.tensor_add(
                                stg[:, sl * 512:(sl + 1) * 512], ps[:],
                                bias_sb[:, d, sl * 512:(sl + 1) * 512])
                        nc.sync.dma_start(zin_d.ap()[d, tb], stg[:])

            # ---- P2..P4 pool (reuses the xg region)
            p2pool = tc.alloc_tile_pool(name="p2", bufs=2)
            ohm_sb = p2pool.tile([T, NTOK], BF16, name="ohm_sb", bufs=1)
            nc.sync.dma_start(ohm_sb[:], ohm[:])
            vm_sb = p2pool.tile([T, NTOK], BF16, name="vm_sb", bufs=1)
            nc.sync.dma_start(vm_sb[:], vmask[:])

            # ---- P2: LSTM steps (fwd and bwd as separate chains)
            c_st = {}
            for d in range(2):
                c_st[d] = spool.tile([BC, HD], F32, tag=f"c{d}", name=f"c{d}")
                nc.sync.dma_start(c_st[d][:], c0.ap()[d * BC:(d + 1) * BC, :])

            zwin = {0: [None] * (S // 4), 1: [None] * (S // 4)}
            for t in range(S):
                ch = t // 4
                ro = t % 4
                for d in range(2):
                    if ro == 0:
                        zw = p2pool.tile([BC, 4, G4], BF16, tag=f"zw{d}", name=f"zw{d}")
                        nc.sync.dma_start(
                            zw[:],
                            zin_d.ap()[d, ch].rearrange("(s b) g -> b s g", s=4))
                        zwin[d][ch] = zw
                    zw = zwin[d][ch]

                    z_ps = zups.tile([BC, G4], F32, tag=f"zps{d}")
                    for sl in range(2):
                        gsl = slice(sl * 512, (sl + 1) * 512)
                        nc.tensor.matmul(
                            z_ps[:, gsl], ident_sb[0:BC, 0:32],
                            zw[:, ro, gsl], start=True, stop=False)
                        for k in range(2):
                            if t == 0:
                                hk = h0t_sb[:, k, d * BC:(d + 1) * BC]
                            elif d == 0:
                                hk = hts[0][:, k, (t - 1) * BC:t * BC]
                            else:
                                # bwd h_{t-1} lives at original pos S-1-(t-1)
                                hk = hts[1][:, k, (S - t) * BC:(S - t + 1) * BC]
                            nc.tensor.matmul(
                                z_ps[:, gsl], hk,
                                whh_sb[:, d, k, gsl],
                                start=False, stop=(k == 1))

                    cell = wpool.tile([BC, 1792], BF16, tag=f"cell{d}",
                                      name=f"cell{d}", bufs=3)
                    sig = cell[:, 0:768]
                    tg = cell[:, 768:G4]
                    t1 = cell[:, G4:G4 + HD]
                    th = cell[:, G4 + HD:G4 + 2 * HD]
                    h_sb = cell[:, G4 + 2 * HD:G4 + 3 * HD]
                    nc.scalar.activation(sig, z_ps[:, 0:768], AF.Sigmoid)
                    nc.scalar.activation(tg, z_ps[:, 768:G4], AF.Tanh)
                    nc.vector.tensor_mul(t1, sig[:, 0:HD], tg)
                    c_old = c_st[d]
                    c_st[d] = spool.tile([BC, HD], F32, tag=f"c{d}", name=f"c{d}")
                    nc.vector.tensor_mul(c_st[d][:], sig[:, HD:2 * HD], c_old[:])
                    nc.vector.tensor_add(c_st[d][:], c_st[d][:], t1)
                    nc.scalar.activation(th, c_st[d][:], AF.Tanh)
                    nc.vector.tensor_mul(h_sb, sig[:, 2 * HD:768], th)

                    # transpose h -> hT (feature-major) into the hT buffer
                    col = (t if d == 0 else S - 1 - t) * BC
                    for k in range(2):
                        tps = smps.tile([128, BC], BF16, tag="sm")
                        nc.tensor.transpose(
                            tps[:], h_sb[:, k * 128:(k + 1) * 128],
                            ident_sb[0:BC, 32:64])
                        if (d + k) % 2 == 0:
                            nc.scalar.copy(hts[d][:, k, col:col + BC], tps[:])
                        else:
                            nc.vector.tensor_copy(hts[d][:, k, col:col + BC],
                                                  tps[:])

            # ---- P3: emissions
            em_accs = []
            for tb in range(NTOK // 512):        # 8 blocks
                blk = slice(tb * 512, (tb + 1) * 512)
                ps = mmps.tile([T, 512], F32, tag="mm")
                for k in range(2):
                    nc.tensor.matmul(ps[:], wout_sb[:, k, :],
                                     hts[0][:, k, blk],
                                     start=(k == 0), stop=False)
                for k in range(2):
                    nc.tensor.matmul(ps[:], wout_sb[:, 2 + k, :],
                                     hts[1][:, k, blk],
                                     start=False, stop=(k == 1))
                acc = wpool.tile([T, 1], F32, tag="emacc" + str(tb), bufs=1, name=f"emacc{tb}")
                scr = wpool.tile([T, 512], F32, tag="ttrscr")
                nc.vector.tensor_mul(scr[:], ps[:], ohm_sb[:, blk])
                nc.vector.tensor_reduce(acc[:], scr[:],
                                        axis=mybir.AxisListType.X, op=ALU.add)
                em_accs.append(acc)
                nc.scalar.copy(em_sb[0:T, blk], ps[:])

            # exp(em + b_out) in place; first 32 cols also get start_trans
            bstart = wpool.tile([T, 1], F32, tag="bstart")
            nc.vector.tensor_add(bstart[:], tab_sb[:, 78:79], tab_sb[:, 76:77])
            nc.scalar.activation(em_sb[0:T, 0:BC], em_sb[0:T, 0:BC],
                                 AF.Exp, bias=bstart[:])
            nc.scalar.activation(em_sb[0:T, BC:512], em_sb[0:T, BC:512],
                                 AF.Exp, bias=tab_sb[:, 78:79])
            for tb in range(1, NTOK // 512):
                blk = slice(tb * 512, (tb + 1) * 512)
                nc.scalar.activation(em_sb[0:T, blk], em_sb[0:T, blk],
                                     AF.Exp, bias=tab_sb[:, 78:79])
            # zero padded positions (rows 0:76); absorber row from host
            for tb in range(NTOK // 512):
                blk = slice(tb * 512, (tb + 1) * 512)
                nc.vector.tensor_mul(em_sb[0:T, blk], em_sb[0:T, blk],
                                     vm_sb[:, blk])
            nc.sync.dma_start(em_sb[T:TA, :], padrow[:])

            # ---- P4: CRF forward in scaled linear space
            mp_sb = cpool.tile([TA, TA], F32)
            nc.scalar.activation(mp_sb[0:T, 0:T], tab_sb[:, 0:T], AF.Exp,
                                 bias=tab_sb[:, 79:80])
            nc.scalar.activation(mp_sb[0:T, T:TA], tab_sb[:, 77:78], AF.Exp,
                                 bias=tab_sb[:, 79:80])
            nc.sync.dma_start(mp_sb[T:TA, 0:TA], absrow.ap()[:, 0:TA])
            eend_sb = cpool.tile([TA, 1], F32)
            nc.scalar.activation(eend_sb[0:T, :], tab_sb[:, 77:78], AF.Exp)
            nc.sync.dma_start(eend_sb[T:TA, :], absrow.ap()[:, 77:78])

            a_prev = em_sb[0:TA, 0:BC]
            for t in range(1, S):
                aps = smps.tile([TA, BC], F32, tag="sm")
                nc.tensor.matmul(aps[:, 0:BC], mp_sb[:], a_prev,
                                 start=True, stop=True)
                a_new = spool.tile([TA, BC], F32, tag="a")
                nc.vector.tensor_mul(a_new[:], aps[:, 0:BC],
                                     em_sb[0:TA, t * BC:(t + 1) * BC])
                a_prev = a_new[:]

            sps = smps.tile([1, BC], F32, tag="sm")
            nc.tensor.matmul(sps[:, 0:BC], eend_sb[:], a_prev,
                             start=True, stop=True)
            logs = wpool.tile([1, BC], F32, tag="logs")
            nc.scalar.activation(logs[:], sps[:, 0:BC], AF.Ln)
            logsum = wpool.tile([1, 1], F32, tag="logsum")
            nc.vector.tensor_reduce(logsum[:], logs[:],
                                    axis=mybir.AxisListType.X, op=ALU.add)

            # gold score: table part
            gacc = wpool.tile([T, 1], F32, tag="gacc")
            scr2 = wpool.tile([T, 79], F32, tag="scr2")
            nc.vector.tensor_mul(scr2[:], gcnt_sb[:], tab_sb[:, 0:79])
            nc.vector.tensor_reduce(gacc[:], scr2[:],
                                    axis=mybir.AxisListType.X, op=ALU.add)
            tot = wpool.tile([T, 1], F32, tag="tot")
            nc.vector.tensor_add(tot[:], gacc[:], em_accs[0][:])
            for acc in em_accs[1:]:
                nc.vector.tensor_add(tot[:], tot[:], acc[:])
            ones = cpool.tile([T, 1], F32)
            nc.vector.memset(ones[:], 1.0)
            scps = smps.tile([1, 1], F32, tag="sm")
            nc.tensor.matmul(scps[:, 0:1], tot[:], ones[:],
                             start=True, stop=True)

            res = wpool.tile([1, 2], F32, tag="res")
            nc.vector.tensor_copy(res[:, 0:1], logsum[:])
            nc.vector.tensor_copy(res[:, 1:2], scps[:, 0:1])
            nc.sync.dma_start(out_d[:], res[:])
            p2pool.release()

    return nc


# ---------------------------------------------------------------- host side
def _gate_perm():
    """PyTorch gate order i,f,g,o -> reordered i,f,o,g (rows of W/b)."""
    return np.concatenate([
        np.arange(0, HD),            # i
        np.arange(HD, 2 * HD),       # f
        np.arange(3 * HD, 4 * HD),   # o
        np.arange(2 * HD, 3 * HD),   # g
    ])


def _pack_w_kxg(w, perm, nchunks):
    """w: [G4, kdim] -> [128, nchunks, G4] bf16, [p, c, g] = w[perm[g], c*128+p]."""
    wp = np.asarray(w)[perm, :]
    out = np.empty((128, nchunks, G4), dtype=ml_dtypes.bfloat16)
    for c in range(nchunks):
        out[:, c, :] = wp[:, c * 128:(c + 1) * 128].T.astype(ml_dtypes.bfloat16)
    return out


def _pack_idx(flat_ids):
    """flat token ids [NTOK] -> int16 [128, NTOK//16] wrap-16 layout."""
    out = np.zeros((128, NTOK // 16), dtype=np.int16)
    out[:16, :] = flat_ids.astype(np.int16).reshape(NTOK // 16, 16).T
    return out


def prep_inputs(inputs):
    """Build per-core input maps + host constants."""
    ids = np.asarray(inputs["input_ids"])
    tags = np.asarray(inputs["tag_ids"])
    lengths = np.asarray(inputs["lengths"])
    perm = _gate_perm()

    embed_bf = np.asarray(inputs["embed_table"]).astype(ml_dtypes.bfloat16)
    def gather_xt(flat_ids):
        g = embed_bf[flat_ids]                       # [NTOK, E] bf16
        return np.ascontiguousarray(
            g.reshape(NTOK, 4, 128).transpose(2, 1, 0))
    wih_pack = np.stack([_pack_w_kxg(inputs["W_ih_f"], perm, 4),
                         _pack_w_kxg(inputs["W_ih_b"], perm, 4)])
    whh_pack = np.stack([_pack_w_kxg(inputs["W_hh_f"], perm, 2),
                         _pack_w_kxg(inputs["W_hh_b"], perm, 2)])
    wo = np.asarray(inputs["W_out"])          # [T, H]
    wout_pack = np.empty((128, 4, T), dtype=ml_dtypes.bfloat16)
    for k in range(4):
        wout_pack[:, k, :] = wo[:, k * 128:(k + 1) * 128].T.astype(
            ml_dtypes.bfloat16)
    bias_f = (np.asarray(inputs["b_ih_f"]) + np.asarray(inputs["b_hh_f"]))[perm]
    bias_b = (np.asarray(inputs["b_ih_b"]) + np.asarray(inputs["b_hh_b"]))[perm]
    biasr = np.stack([np.broadcast_to(bias_f, (128, G4)),
                      np.broadcast_to(bias_b, (128, G4))]).astype(
                          ml_dtypes.bfloat16)

    ident = np.zeros((128, 96), dtype=ml_dtypes.bfloat16)
    for p in range(128):
        ident[p, p % 32] = 1
    for p in range(BC):
        ident[p, 32 + p] = 1

    trans = np.asarray(inputs["trans"]).astype(np.float64)
    kappa = float(np.log(np.exp(trans).sum(axis=0).mean()))
    tables = np.zeros((T, 80), dtype=np.float32)
    tables[:, 0:T] = trans.astype(np.float32)
    tables[:, 76] = np.asarray(inputs["start_trans"])
    tables[:, 77] = np.asarray(inputs["end_trans"])
    tables[:, 78] = np.asarray(inputs["b_out"])
    tables[:, 79] = -kappa

    h0 = np.asarray(inputs["h0"])             # [2, B, HD]
    c0 = np.asarray(inputs["c0"])

    in_maps = []
    k_len_total = 0
    for c in range(N_CORES):
        bs = slice(c * BC, (c + 1) * BC)
        ids_c = ids[bs]
        tags_c = tags[bs]
        len_c = lengths[bs].astype(np.int64)
        k_len_total += int(np.minimum(len_c, S - 1).sum())

        idx_f = ids_c.T.reshape(-1)                    # token (s, b) order
        idx_b = ids_c[:, ::-1].T.reshape(-1)
        xt = np.stack([gather_xt(idx_f), gather_xt(idx_b)])

        svec = np.arange(S)[None, :]
        valid = (svec < len_c[:, None]).T.reshape(-1)  # [(s, b)]
        ohm = np.zeros((T, NTOK), dtype=ml_dtypes.bfloat16)
        tt = tags_c.T.reshape(-1)
        pos = np.arange(NTOK)
        ohm[tt[valid], pos[valid]] = 1
        vm = np.broadcast_to(valid.astype(ml_dtypes.bfloat16),
                             (T, NTOK)).copy()
        padr = (~valid).astype(np.float32)[None, :]

        Cm = np.zeros((T, T), dtype=np.float32)
        h0v = np.zeros(T, dtype=np.float32)
        hLv = np.zeros(T, dtype=np.float32)
        for b in range(BC):
            L = int(len_c[b])
            tg = tags_c[b, :L]
            np.add.at(Cm, (tg[:-1], tg[1:]), 1)
            h0v[tg[0]] += 1
            hLv[tg[-1]] += 1
        nv = ohm.astype(np.float32).sum(axis=1)
        gcnt = np.concatenate([Cm, h0v[:, None], hLv[:, None], nv[:, None]],
                              axis=1)

        h0t = np.zeros((128, 2, 2 * BC), dtype=ml_dtypes.bfloat16)
        for k in range(2):
            h0t[:, k, 0:BC] = h0[0][bs][:, k * 128:(k + 1) * 128].T
            h0t[:, k, BC:2 * BC] = h0[1][bs][:, k * 128:(k + 1) * 128].T
        c0c = np.concatenate([c0[0][bs], c0[1][bs]], axis=0).astype(np.float32)

        absrow = np.zeros((1, 80), dtype=np.float32)
        absrow[0, 76] = 1.0
        absrow[0, 77] = 1.0
        in_maps.append(dict(
            xt=xt, wih=wih_pack, whh=whh_pack,
            wout=wout_pack, biasr=biasr, h0t=h0t, c0=c0c, ident=ident,
            tables=tables, gcnt=gcnt.astype(np.float32), ohm=ohm,
            vmask=vm, padrow=padr, absrow=absrow,
        ))

    return in_maps, dict(kappa=kappa, k_len_total=k_len_total)


def finalize(results, host):
    logz = sum(float(r["out"][0, 0]) for r in results)
    score = sum(float(r["out"][0, 1]) for r in results)
    logz += host["kappa"] * host["k_len_total"]
    return np.float32((logz - score) / B)


# ---------------------------------------------------------------- entry point
_COMPILED = {}


def kernel(**inputs):
    """Full-input BiLSTM-CRF loss on 8 NeuronCores (data parallel)."""
    from concourse.bass_utils import run_bass_kernel_spmd
    in_maps, host = prep_inputs(inputs)
    if "nc" not in _COMPILED:
        _COMPILED["nc"] = build_nc()
    nc = _COMPILED["nc"]
    res = run_bass_kernel_spmd(nc, in_maps, core_ids=list(range(N_CORES)))
    return np.asarray(finalize(res.results, host))



# revision 9
# speedup vs baseline: 2.1386x; 2.1386x over previous
"""BiLSTM-CRF loss kernel for Trainium2, 8-core data parallel.

Transposed-gate design: LSTM gates live on PARTITIONS (8 chunks of 128),
batch (32) on the free dim. Benefits vs the batch-on-partitions layout:
  - every Act/DVE op uses all 128 partitions (4x fewer cycles),
  - h is produced feature-major, so the per-step PE transposes and copies
    disappear (h feeds the next step's matmul and the emission matmuls
    directly),
  - the input projection x@W_ih is fused into the step loop as PSUM
    accumulation (no DRAM round-trip for zin),
  - cell-state elementwise math runs in bf16 SBUF (DVE 4x perf mode).
Per core (batch shard of 32): fwd/bwd chains are independent instruction
streams that pipeline across engines; emissions are interleaved into the
step loop as soon as both directions have produced the needed columns;
CRF forward pass in scaled linear space with an absorbing 77th tag runs
as a tail (em col 0 is only ready after the last bwd step).
Host combines the 8 per-core partial sums into the scalar loss.
"""

import numpy as np
import ml_dtypes

import concourse.bass as bass
import concourse.mybir as mybir
from concourse.tile import TileContext
from concourse.vector_clock import ScopedClock

N_CORES = 8
B, S, E, HD, T, V = 256, 128, 512, 256, 76, 30000
BC = B // N_CORES          # 32 batch per core
G4 = 4 * HD                # 1024 gates
TA = T + 1                 # 77 tags with absorber
NTOK = S * BC              # 4096 tokens per direction per core
NGC = 8                    # gate chunks of 128 (i,i,f,f,o,o,g,g after perm)
NEC = 4                    # embed chunks of 128

dt = mybir.dt
F32, BF16 = dt.float32, dt.bfloat16
AF = mybir.ActivationFunctionType
ALU = mybir.AluOpType

# ---------------------------------------------------------------- tile patch
# This walrus build rejects >1 sem wait on CTRL-class (Drain/NoOp)
# instructions; split the Tile tail-drain waits across preceding NOPs.
_MAX_WAITS = 1

_WAIT_LIMITS = {}


def _split_excess_waits(nc):
    """Non-DMA instructions accept only one sem wait on this walrus build;
    move excess waits onto NOPs spliced in front (same engine, same order)."""
    for f in nc.m.functions:
        stack = list(f.blocks)
        while stack:
            bb = stack.pop()
            for sub in getattr(bb, "blocks", []) or []:
                stack.append(sub)
            insts = getattr(bb, "instructions", None)
            if not insts:
                continue
            newlist = []
            changed = False
            for inst in insts:
                si = inst.sync_info
                lim = _WAIT_LIMITS.get(type(inst).__name__, 1)
                if si is not None and si.on_wait and len(si.on_wait) > lim:
                    waits = list(si.on_wait)
                    si.on_wait = waits[-lim:]
                    for w in waits[:-lim]:
                        nop = mybir.InstNoOp(
                            name=f"I-wsplit{nc.next_id()}", ins=[], outs=[],
                            engine=inst.engine,
                            sync_info=mybir.SyncInfo(on_wait=[w], on_update=[]),
                        )
                        newlist.append(nop)
                    changed = True
                newlist.append(inst)
            if changed:
                insts[:] = newlist


def _patched_drain_and_barrier(self, tick_clock, wait_clock):
    nc = self.nc
    _split_excess_waits(nc)
    nops = [nc.sync.nop(nofuse=True, hint=f"waitsplit{i}") for i in range(16)]
    drain_inst = nc.sync.drain()
    wait_clock.add_sem_waits(
        drain_inst.ins, ScopedClock({None: tick_clock.global_clock})
    )
    si = drain_inst.ins.sync_info
    if si is not None and si.on_wait and len(si.on_wait) > _MAX_WAITS:
        waits = list(si.on_wait)
        chunks = [waits[i:i + _MAX_WAITS] for i in range(0, len(waits), _MAX_WAITS)]
        si.on_wait = chunks[-1]
        assert len(chunks) - 1 <= len(nops), "too many wait chunks"
        for i, ch in enumerate(chunks[:-1]):
            ni = nops[i].ins
            if ni.sync_info is None:
                ni.sync_info = mybir.SyncInfo(on_wait=ch, on_update=[])
            else:
                ni.sync_info.on_wait = list(ni.sync_info.on_wait) + ch
    nc.all_engine_barrier()
    assert self.sems is not None
    popped = nc._tile_sem_poison_stack.pop()
    assert popped is self._sem_poison
    allsems = list(self.sems.allocated().values())
    for i in range(0, len(allsems), 8):
        nc.clear_and_free_semaphores(allsems[i:i + 8])
    nc.all_engine_barrier()


def apply_tile_patch():
    TileContext._drain_and_barrier = _patched_drain_and_barrier


# ---------------------------------------------------------------- builder
def build_nc():
    apply_tile_patch()
    nc = bass.Bass("TRN2", target_bir_lowering=False, debug=False,
                   num_devices=N_CORES)

    xt_d = nc.dram_tensor("xt", [2, 128, NEC, NTOK], BF16, kind="ExternalInput")
    wiht = nc.dram_tensor("wiht", [128, 2, NEC, NGC, 128], BF16,
                          kind="ExternalInput")
    whht = nc.dram_tensor("whht", [128, 2, 2, NGC, 128], BF16,
                          kind="ExternalInput")
    wout = nc.dram_tensor("wout", [128, 4, T], BF16, kind="ExternalInput")
    biasl = nc.dram_tensor("biasl", [NGC, 2, 128], BF16, kind="ExternalInput")
    bdelta = nc.dram_tensor("bdelta", [NGC, NGC * BC], BF16,
                            kind="ExternalInput")
    h0t = nc.dram_tensor("h0t", [128, 2, 2 * BC], BF16, kind="ExternalInput")
    c0t = nc.dram_tensor("c0t", [128, 2, 2 * BC], BF16, kind="ExternalInput")
    # tables: [trans(0:76) | start(76) | end(77) | bout(78) | negkappa(79)]
    tables = nc.dram_tensor("tables", [T, 80], F32, kind="ExternalInput")
    gcnt = nc.dram_tensor("gcnt", [T, 79], F32, kind="ExternalInput")
    ohm = nc.dram_tensor("ohm", [T, NTOK], BF16, kind="ExternalInput")
    vmask = nc.dram_tensor("vmask", [T, NTOK], BF16, kind="ExternalInput")
    padrow = nc.dram_tensor("padrow", [1, NTOK], F32, kind="ExternalInput")
    absrow = nc.dram_tensor("absrow", [1, 80], F32, kind="ExternalInput")
    out_d = nc.dram_tensor("out", [1, 2], F32, kind="ExternalOutput")

    with TileContext(nc) as tc:
        with (
            tc.tile_pool(name="const", bufs=1) as cpool,
            tc.tile_pool(name="hbuf", bufs=1) as hpool,
            tc.tile_pool(name="work", bufs=2) as wpool,
            tc.tile_pool(name="state", bufs=2) as spool,
        ):
            # ---- weights / small constants
            wih_sb = cpool.tile([128, 2, NEC, NGC, 128], BF16)
            nc.sync.dma_start(wih_sb[:], wiht[:])
            whh_sb = cpool.tile([128, 2, 2, NGC, 128], BF16)
            nc.sync.dma_start(whh_sb[:], whht[:])
            wout_sb = cpool.tile([128, 4, T], BF16)
            nc.sync.dma_start(wout_sb[:], wout[:])
            biasl_sb = cpool.tile([NGC, 2, 128], BF16)
            nc.sync.dma_start(biasl_sb[:], biasl[:])
            bdelta_sb = cpool.tile([NGC, NGC * BC], BF16)
            nc.sync.dma_start(bdelta_sb[:], bdelta[:])
            h0t_sb = cpool.tile([128, 2, 2 * BC], BF16)
            nc.sync.dma_start(h0t_sb[:], h0t[:])
            c0t_sb = cpool.tile([128, 2, 2 * BC], BF16)
            nc.sync.dma_start(c0t_sb[:], c0t[:])
            tab_sb = cpool.tile([T, 80], F32)
            nc.sync.dma_start(tab_sb[:], tables[:])
            gcnt_sb = cpool.tile([T, 79], F32)
            nc.sync.dma_start(gcnt_sb[:], gcnt[:])

            # ---- big persistent buffers
            xg = {0: hpool.tile([128, NEC, NTOK], BF16, name="xg0"),
                  1: hpool.tile([128, NEC, NTOK], BF16, name="xg1")}
            hts = {0: hpool.tile([128, 2, NTOK], BF16, name="hft"),
                   1: hpool.tile([128, 2, NTOK], BF16, name="hbt")}
            em_sb = hpool.tile([TA, NTOK], F32, name="em_sb")
            ohm_sb = hpool.tile([T, NTOK], BF16, name="ohm_sb")
            vm_sb = hpool.tile([T, NTOK], BF16, name="vm_sb")

            # token stream DMAs, interleaved across directions so both
            # chains' early steps have data promptly
            XCH = 512
            for c in range(NTOK // XCH):
                cs = slice(c * XCH, (c + 1) * XCH)
                for d in range(2):
                    nc.sync.dma_start(xg[d][:, :, cs], xt_d.ap()[d, :, :, cs])
            nc.sync.dma_start(ohm_sb[:], ohm[:])
            nc.sync.dma_start(vm_sb[:], vmask[:])
            nc.sync.dma_start(em_sb[T:TA, :], padrow[:])

            # ---- CRF constants (absorbing 77th tag; scaled linear space)
            mp_sb = cpool.tile([TA, TA], F32)
            nc.scalar.activation(mp_sb[0:T, 0:T], tab_sb[:, 0:T], AF.Exp,
                                 bias=tab_sb[:, 79:80])
            nc.scalar.activation(mp_sb[0:T, T:TA], tab_sb[:, 77:78], AF.Exp,
                                 bias=tab_sb[:, 79:80])
            nc.sync.dma_start(mp_sb[T:TA, 0:TA], absrow.ap()[:, 0:TA])
            eend_sb = cpool.tile([TA, 1], F32)
            nc.scalar.activation(eend_sb[0:T, :], tab_sb[:, 77:78], AF.Exp)
            nc.sync.dma_start(eend_sb[T:TA, :], absrow.ap()[:, 77:78])
            bstart = cpool.tile([T, 1], F32)
            nc.vector.tensor_add(bstart[:], tab_sb[:, 78:79], tab_sb[:, 76:77])

            # ---- PSUM pools for the loop
            zpool = tc.alloc_tile_pool(name="zps", bufs=2, space="PSUM")
            empool = tc.alloc_tile_pool(name="emps", bufs=2, space="PSUM")

            em_accs = []

            def emit_em_block(tb):
                blk = slice(tb * 512, (tb + 1) * 512)
                ps = empool.tile([T, 512], F32, tag="em", name="emps")
                for k in range(2):
                    nc.tensor.matmul(ps[:], wout_sb[:, k, :], hts[0][:, k, blk],
                                     start=(k == 0), stop=False)
                for k in range(2):
                    nc.tensor.matmul(ps[:], wout_sb[:, 2 + k, :],
                                     hts[1][:, k, blk],
                                     start=False, stop=(k == 1))
                # gold-path emission dot (raw em) fused mul+reduce
                acc = wpool.tile([T, 1], F32, tag=f"emacc{tb}", bufs=1,
                                 name=f"emacc{tb}")
                scr = wpool.tile([T, 512], F32, tag="ttrscr", name="ttrscr")
                nc.vector.tensor_mul(scr[:], ps[:], ohm_sb[:, blk])
                nc.vector.tensor_reduce(acc[:], scr[:],
                                        axis=mybir.AxisListType.X, op=ALU.add)
                em_accs.append(acc)
                # exp(em + b_out) into em_sb (+ start_trans on the t=0 cols)
                if tb == 0:
                    nc.scalar.activation(em_sb[0:T, 0:BC], ps[:, 0:BC],
                                         AF.Exp, bias=bstart[:])
                    nc.scalar.activation(em_sb[0:T, BC:512], ps[:, BC:512],
                                         AF.Exp, bias=tab_sb[:, 78:79])
                else:
                    nc.scalar.activation(em_sb[0:T, blk], ps[:],
                                         AF.Exp, bias=tab_sb[:, 78:79])
                # zero padded positions (rows 0:76)
                nc.vector.tensor_mul(em_sb[0:T, blk], em_sb[0:T, blk],
                                     vm_sb[:, blk])

            # emission blocks become ready mid-loop once both directions
            # have written the block's columns
            em_sched = {80: [3, 4], 96: [2, 5], 112: [1, 6]}

            # ---- LSTM step loop (transposed gates: z[g_chunk, batch])
            WHH_ORDER = [6, 7, 0, 1, 2, 3, 4, 5]   # g gates first -> early tanh

            def emit_xproj(t, d, ztile):
                # bias broadcast into all 8 chunks, then x @ W_ih accumulation
                nc.tensor.matmul(ztile[:, :, :], biasl_sb[:, d, :],
                                 bdelta_sb[:, :], start=True, stop=False)
                tcol = slice(t * BC, (t + 1) * BC)
                for gc in range(NGC):
                    for ec in range(NEC):
                        nc.tensor.matmul(ztile[:, gc, :],
                                         wih_sb[:, d, ec, gc, :],
                                         xg[d][:, ec, tcol],
                                         start=False, stop=False)

            z_cur = {}
            for d in range(2):
                z_cur[d] = zpool.tile([128, NGC, BC], F32, tag=f"z{d}",
                                      name=f"z{d}")
                emit_xproj(0, d, z_cur[d])

            c_st = {0: c0t_sb[:, :, 0:BC], 1: c0t_sb[:, :, BC:2 * BC]}

            for t in range(S):
                z_nxt = {}
                for d in range(2):
                    # recurrent part: h_{t-1} @ W_hh into the same PSUM tile
                    z = z_cur[d]
                    for gc in WHH_ORDER:
                        for k in range(2):
                            if t == 0:
                                hk = h0t_sb[:, k, d * BC:(d + 1) * BC]
                            elif d == 0:
                                hk = hts[0][:, k, (t - 1) * BC:t * BC]
                            else:
                                hk = hts[1][:, k, (S - t) * BC:(S - t + 1) * BC]
                            nc.tensor.matmul(z[:, gc, :],
                                             whh_sb[:, d, k, gc, :], hk,
                                             start=False, stop=(k == 1))
                # prefetch next step's input projection while this step's
                # activations run
                if t + 1 < S:
                    for d in range(2):
                        z_nxt[d] = zpool.tile([128, NGC, BC], F32, tag=f"z{d}",
                                              name=f"z{d}")
                        emit_xproj(t + 1, d, z_nxt[d])

                for d in range(2):
                    z = z_cur[d]
                    tg = wpool.tile([128, 2, BC], BF16, tag=f"tg{d}",
                                    name=f"tg{d}")
                    nc.scalar.activation(tg[:], z[:, 6:8, :], AF.Tanh)
                    sg = wpool.tile([128, 6, BC], BF16, tag=f"sg{d}",
                                    name=f"sg{d}")
                    nc.scalar.activation(sg[:], z[:, 0:6, :], AF.Sigmoid)

                    c_old = c_st[d]
                    c_new = spool.tile([128, 2, BC], BF16, tag=f"c{d}",
                                       name=f"c{d}")
                    t1 = wpool.tile([128, 2, BC], BF16, tag=f"t1{d}",
                                    name=f"t1{d}")
                    nc.vector.tensor_mul(c_new[:], sg[:, 2:4, :], c_old)
                    nc.vector.tensor_mul(t1[:], sg[:, 0:2, :], tg[:])
                    nc.vector.tensor_add(c_new[:], c_new[:], t1[:])
                    th = wpool.tile([128, 2, BC], BF16, tag=f"th{d}",
                                    name=f"th{d}")
                    nc.scalar.activation(th[:], c_new[:], AF.Tanh)
                    col = (t if d == 0 else S - 1 - t) * BC
                    nc.vector.tensor_mul(hts[d][:, :, col:col + BC],
                                         sg[:, 4:6, :], th[:])
                    c_st[d] = c_new[:]
                z_cur = z_nxt

                for tb in em_sched.get(t + 1, []):
                    emit_em_block(tb)

            emit_em_block(0)
            emit_em_block(7)
            empool.release()
            zpool.release()

            # ---- CRF forward pass (sequential over time, batch on free dim)
            crfpool = tc.alloc_tile_pool(name="crfps", bufs=2, space="PSUM")
            a_prev = em_sb[0:TA, 0:BC]
            for t in range(1, S):
                aps = crfpool.tile([TA, BC], F32, tag="crf", name="aps")
                nc.tensor.matmul(aps[:], mp_sb[:], a_prev,
                                 start=True, stop=True)
                a_new = spool.tile([TA, BC], F32, tag="a", name="a_new")
                nc.vector.tensor_mul(a_new[:], aps[:],
                                     em_sb[0:TA, t * BC:(t + 1) * BC])
                a_prev = a_new[:]

            sps = crfpool.tile([1, BC], F32, tag="crfs", bufs=1, name="sps")
            nc.tensor.matmul(sps[:], eend_sb[:], a_prev, start=True, stop=True)
            logs = wpool.tile([1, BC], F32, tag="logs", name="logs")
            nc.scalar.activation(logs[:], sps[:], AF.Ln)
            logsum = wpool.tile([1, 1], F32, tag="logsum", name="logsum")
            nc.vector.tensor_reduce(logsum[:], logs[:],
                                    axis=mybir.AxisListType.X, op=ALU.add)

            # gold score: transition/start/end table part via counts
            gacc = wpool.tile([T, 1], F32, tag="gacc", name="gacc")
            scr2 = wpool.tile([T, 79], F32, tag="scr2", name="scr2")
            nc.vector.tensor_mul(scr2[:], gcnt_sb[:], tab_sb[:, 0:79])
            nc.vector.tensor_reduce(gacc[:], scr2[:],
                                    axis=mybir.AxisListType.X, op=ALU.add)
            tot = wpool.tile([T, 1], F32, tag="tot", name="tot")
            nc.vector.tensor_add(tot[:], gacc[:], em_accs[0][:])
            for acc in em_accs[1:]:
                nc.vector.tensor_add(tot[:], tot[:], acc[:])
            ones = cpool.tile([T, 1], F32)
            nc.vector.memset(ones[:], 1.0)
            scps = crfpool.tile([1, 1], F32, tag="crfsc", bufs=1, name="scps")
            nc.tensor.matmul(scps[:], tot[:], ones[:], start=True, stop=True)

            res = wpool.tile([1, 2], F32, tag="res", name="res")
            nc.vector.tensor_copy(res[:, 0:1], logsum[:])
            nc.vector.tensor_copy(res[:, 1:2], scps[:])
            nc.sync.dma_start(out_d[:], res[:])
            crfpool.release()

    return nc


# ---------------------------------------------------------------- host side
def _gate_perm():
    """PyTorch gate order i,f,g,o -> reordered i,f,o,g (rows of W/b)."""
    return np.concatenate([
        np.arange(0, HD),            # i
        np.arange(HD, 2 * HD),       # f
        np.arange(3 * HD, 4 * HD),   # o
        np.arange(2 * HD, 3 * HD),   # g
    ])


def _pack_w_t(w, perm, nkc):
    """w: [G4, kdim] -> [128, nkc, NGC, 128] bf16 with
    out[k_p, kc, gc, gf] = w[perm[gc*128+gf], kc*128+k_p]."""
    wp = np.asarray(w)[perm, :]                       # [G4, kdim]
    out = np.empty((128, nkc, NGC, 128), dtype=ml_dtypes.bfloat16)
    for kc in range(nkc):
        for gc in range(NGC):
            blk = wp[gc * 128:(gc + 1) * 128, kc * 128:(kc + 1) * 128]
            out[:, kc, gc, :] = blk.T.astype(ml_dtypes.bfloat16)
    return out


def prep_inputs(inputs):
    """Build per-core input maps + host constants."""
    ids = np.asarray(inputs["input_ids"])
    tags = np.asarray(inputs["tag_ids"])
    lengths = np.asarray(inputs["lengths"])
    perm = _gate_perm()

    embed_bf = np.asarray(inputs["embed_table"]).astype(ml_dtypes.bfloat16)

    def gather_xt(flat_ids):
        g = embed_bf[flat_ids]                       # [NTOK, E] bf16
        return np.ascontiguousarray(
            g.reshape(NTOK, NEC, 128).transpose(2, 1, 0))

    wih_pack = np.stack([_pack_w_t(inputs["W_ih_f"], perm, NEC),
                         _pack_w_t(inputs["W_ih_b"], perm, NEC)], axis=1)
    whh_pack = np.stack([_pack_w_t(inputs["W_hh_f"], perm, 2),
                         _pack_w_t(inputs["W_hh_b"], perm, 2)], axis=1)
    wo = np.asarray(inputs["W_out"])          # [T, H]
    wout_pack = np.empty((128, 4, T), dtype=ml_dtypes.bfloat16)
    for k in range(4):
        wout_pack[:, k, :] = wo[:, k * 128:(k + 1) * 128].T.astype(
            ml_dtypes.bfloat16)
    bias_f = (np.asarray(inputs["b_ih_f"]) + np.asarray(inputs["b_hh_f"]))[perm]
    bias_b = (np.asarray(inputs["b_ih_b"]) + np.asarray(inputs["b_hh_b"]))[perm]
    biasl = np.stack([bias_f.reshape(NGC, 128),
                      bias_b.reshape(NGC, 128)], axis=1).astype(
                          ml_dtypes.bfloat16)
    bdelta = np.zeros((NGC, NGC * BC), dtype=ml_dtypes.bfloat16)
    for k in range(NGC):
        bdelta[k, k * BC:(k + 1) * BC] = 1

    trans = np.asarray(inputs["trans"]).astype(np.float64)
    kappa = float(np.log(np.exp(trans).sum(axis=0).mean()))
    tables = np.zeros((T, 80), dtype=np.float32)
    tables[:, 0:T] = trans.astype(np.float32)
    tables[:, 76] = np.asarray(inputs["start_trans"])
    tables[:, 77] = np.asarray(inputs["end_trans"])
    tables[:, 78] = np.asarray(inputs["b_out"])
    tables[:, 79] = -kappa

    h0 = np.asarray(inputs["h0"])             # [2, B, HD]
    c0 = np.asarray(inputs["c0"])

    in_maps = []
    k_len_total = 0
    for c in range(N_CORES):
        bs = slice(c * BC, (c + 1) * BC)
        ids_c = ids[bs]
        tags_c = tags[bs]
        len_c = lengths[bs].astype(np.int64)
        k_len_total += int(np.minimum(len_c, S - 1).sum())

        idx_f = ids_c.T.reshape(-1)                    # token (s, b) order
        idx_b = ids_c[:, ::-1].T.reshape(-1)
        xt = np.stack([gather_xt(idx_f), gather_xt(idx_b)])

        svec = np.arange(S)[None, :]
        valid = (svec < len_c[:, None]).T.reshape(-1)  # [(s, b)]
        ohm = np.zeros((T, NTOK), dtype=ml_dtypes.bfloat16)
        tt = tags_c.T.reshape(-1)
        pos = np.arange(NTOK)
        ohm[tt[valid], pos[valid]] = 1
        vm = np.broadcast_to(valid.astype(ml_dtypes.bfloat16),
                             (T, NTOK)).copy()
        padr = (~valid).astype(np.float32)[None, :]

        Cm = np.zeros((T, T), dtype=np.float32)
        h0v = np.zeros(T, dtype=np.float32)
        hLv = np.zeros(T, dtype=np.float32)
        for b in range(BC):
            L = int(len_c[b])
            tg = tags_c[b, :L]
            np.add.at(Cm, (tg[:-1], tg[1:]), 1)
            h0v[tg[0]] += 1
            hLv[tg[-1]] += 1
        nv = ohm.astype(np.float32).sum(axis=1)
        gcnt = np.concatenate([Cm, h0v[:, None], hLv[:, None], nv[:, None]],
                              axis=1)

        h0t = np.zeros((128, 2, 2 * BC), dtype=ml_dtypes.bfloat16)
        c0t = np.zeros((128, 2, 2 * BC), dtype=ml_dtypes.bfloat16)
        for k in range(2):
            for d in range(2):
                h0t[:, k, d * BC:(d + 1) * BC] = \
                    h0[d][bs][:, k * 128:(k + 1) * 128].T
                c0t[:, k, d * BC:(d + 1) * BC] = \
                    c0[d][bs][:, k * 128:(k + 1) * 128].T

        absrow = np.zeros((1, 80), dtype=np.float32)
        absrow[0, 76] = 1.0
        absrow[0, 77] = 1.0
        in_maps.append(dict(
            xt=xt, wiht=wih_pack, whht=whh_pack, wout=wout_pack,
            biasl=biasl, bdelta=bdelta, h0t=h0t, c0t=c0t,
            tables=tables, gcnt=gcnt.astype(np.float32), ohm=ohm,
            vmask=vm, padrow=padr, absrow=absrow,
        ))

    return in_maps, dict(kappa=kappa, k_len_total=k_len_total)


def finalize(results, host):
    logz = sum(float(r["out"][0, 0]) for r in results)
    score = sum(float(r["out"][0, 1]) for r in results)
    logz += host["kappa"] * host["k_len_total"]
    return np.float32((logz - score) / B)


# ---------------------------------------------------------------- entry point
_COMPILED = {}


def kernel(**inputs):
    """Full-input BiLSTM-CRF loss on 8 NeuronCores (data parallel)."""
    from concourse.bass_utils import run_bass_kernel_spmd
    in_maps, host = prep_inputs(inputs)
    if "nc" not in _COMPILED:
        _COMPILED["nc"] = build_nc()
    nc = _COMPILED["nc"]
    res = run_bass_kernel_spmd(nc, in_maps, core_ids=list(range(N_CORES)))
    return np.asarray(finalize(res.results, host))
